# revision 10
# baseline (speedup 1.0000x reference)
"""Trainium2 Bass kernel for nn_CellsExtractor (axial attention over ragged grid).

16 batches data-parallel over 8 cores (2 per core, processed sequentially,
x resident in SBUF). Per batch, 4 stages: col-attn(h0), row-attn(h0),
col-attn(h1), row-attn(h1). Each stage = 96 sequences x 96 positions,
C=256, 4 heads of D=64.

Per stage:
  k,q: fp32r projections (weights stationary), psum->sbuf bf16 copies with
       per-partition bias (k on ACT-diverted DVE path, q on DVE).
  v^T: per-seq matmuls with x-slice stationary -> [pos, c] layout directly;
       v-bias folded in via a K=1 accumulating matmul; plain ACT Copy out.
  scores^T (lhsT=q_h, rhs=k_h) -> [m,n] psum, heads row-packed (0/64),
       bank-aligned psum slots.
  e = exp(scores/8 + maskbias[m]) via one ACT op per (seq, head-pair),
       written bf16 into a ring tile with a 97th eps row (1e-30).
  S (softmax denominators broadcast to all partitions) via ones-matrix
       matmuls, col-packed head pair, 2 seqs per bank.
  AV (lhsT = v^T head columns, rhs = e) -> [c, n] psum, col-packed pair.
  D = AV * reciprocal(S); x += D on GPSIMD.
Masking via host-precomputed biasmat[m, seq] (0 / -30000); fully-masked
sequences give e=0 everywhere -> D = 0/eps = 0 -> x unchanged (matches ref).
"""
import os
import numpy as np
import ml_dtypes

import concourse.bass as bass
from concourse import bacc
import concourse.tile as tile
from concourse import mybir
from concourse.bass_utils import run_bass_kernel_spmd

F32 = mybir.dt.float32
BF16 = mybir.dt.bfloat16
F32R = mybir.dt.float32r
AF = mybir.ActivationFunctionType
ALU = mybir.AluOpType

BS, C, NR, NC = 16, 256, 96, 96
NCORES = 8
BLOCAL = BS // NCORES
NSEQ, NPOS = 96, 96
CHUNK = 16
NCHUNKS = NSEQ // CHUNK
SUB = 4                       # seqs per projection matmul (N = 4*96 = 384)
STAGES = [(0, 0), (0, 1), (1, 0), (1, 1)]   # (head, typ); typ 0=col, 1=row
NAMES = ["col_k", "col_q", "col_v", "row_k", "row_q", "row_v"]

_BUILT = None
K_PHASE = os.environ.get("K_PHASE", "full")   # dma | proj | attn | full
K_RESID = os.environ.get("K_RESID", "gpsimd") # gpsimd | dve
K_NSTAGE = int(os.environ.get("K_NSTAGE", "8"))


def _f32r(ap):
    return ap.bitcast(F32R)


def _reap(ap, dims):
    """Reorder/select free dims of an AP: dims are indices into ap.ap[1:]."""
    return bass.AP(tensor=ap.tensor, offset=ap.offset,
                   ap=[ap.ap[0]] + [ap.ap[1 + d] for d in dims])


def _build():
    nc = bacc.Bacc()
    feats = nc.dram_tensor("feats", [BLOCAL, C, NR, NC], F32, kind="ExternalInput")
    w_all = nc.dram_tensor("w_all", [12, C, C], F32, kind="ExternalInput")
    b_all = nc.dram_tensor("b_all", [13, C], F32, kind="ExternalInput")
    bm_in = nc.dram_tensor("bm_in", [BLOCAL, 2, 96, 96], F32, kind="ExternalInput")
    out = nc.dram_tensor("out", [BLOCAL, C, NR, NC], F32, kind="ExternalOutput")

    with tile.TileContext(nc) as tc:
        # ---- persistent sbuf tensors ----
        x = nc.alloc_sbuf_tensor("x", [128, 2, NR, NC], F32R).ap()
        e_sb = nc.alloc_sbuf_tensor("e_sb", [97, 8, 192], BF16).ap()
        ones97 = nc.alloc_sbuf_tensor("ones97", [97, 128], BF16).ap()
        Wsb = nc.alloc_sbuf_tensor("Wsb", [128, 12, 2, 256], F32R).ap()
        Bsb = nc.alloc_sbuf_tensor("Bsb", [128, 12, 2], F32).ap()
        Bflat = nc.alloc_sbuf_tensor("Bflat", [1, 13 * 256], F32R).ap()
        biasmat = nc.alloc_sbuf_tensor("biasmat", [96, BLOCAL, 2, 96], F32).ap()

        nc.vector.memset(e_sb[96:97, :, :], 1e-30)
        nc.vector.memset(ones97[:, :], 1.0)

        # weights: w_all[m] is pre-transposed on host to [cin, cout]
        nc.gpsimd.dma_start(
            out=Wsb,
            in_=bass.AP(tensor=w_all, offset=0,
                        ap=[[256, 128], [65536, 12], [128 * 256, 2], [1, 256]]))
        with nc.allow_non_contiguous_dma(reason="3KB bias load, once"):
            nc.sync.dma_start(
                out=Bsb,
                in_=bass.AP(tensor=b_all, offset=0,
                            ap=[[1, 128], [256, 12], [128, 2]]))
        nc.gpsimd.dma_start(
            out=Bflat,
            in_=bass.AP(tensor=b_all, offset=0, ap=[[3328, 1], [1, 3328]]))
        nc.sync.dma_start(
            out=biasmat,
            in_=bass.AP(tensor=bm_in, offset=0,
                        ap=[[96, 96], [2 * 96 * 96, BLOCAL], [96 * 96, 2], [1, 96]]))

        with (
            tc.tile_pool(name="kq_ps", bufs=1, space="PSUM") as kq_ps,
            tc.tile_pool(name="vt_ps", bufs=1, space="PSUM") as vt_ps,
            tc.tile_pool(name="sc_ps", bufs=1, space="PSUM") as sc_ps,
            tc.tile_pool(name="s2_ps", bufs=1, space="PSUM") as s2_ps,
            tc.tile_pool(name="av_ps", bufs=1, space="PSUM") as av_ps,
            tc.tile_pool(name="kq_sb", bufs=2) as kq_sb,
            tc.tile_pool(name="vt_sb", bufs=2) as vt_sbp,
            tc.tile_pool(name="sm_sb", bufs=4) as sm_sb,
        ):
            for b in range(BLOCAL):
                # load x for this batch
                nc.gpsimd.dma_start(
                    out=x,
                    in_=bass.AP(tensor=feats, offset=b * C * NR * NC,
                                ap=[[NR * NC, 128], [128 * NR * NC, 2],
                                    [NC, NR], [1, NC]]))
                for st, (head, typ) in enumerate(STAGES):
                    if b * 4 + st >= K_NSTAGE or K_PHASE == "dma":
                        continue
                    ones1 = Bflat[0:1, 12 * 256:12 * 256 + 128]
                    _emit_stage(nc, tc, b, head, typ,
                                x, e_sb, ones97, ones1, Wsb, Bsb, Bflat, biasmat,
                                kq_ps, vt_ps, sc_ps, s2_ps, av_ps,
                                kq_sb, vt_sbp, sm_sb)
                nc.sync.dma_start(
                    out=bass.AP(tensor=out, offset=b * C * NR * NC,
                                ap=[[NR * NC, 128], [128 * NR * NC, 2],
                                    [NC, NR], [1, NC]]),
                    in_=x.bitcast(F32))
    nc.finalize()
    return nc


def _emit_stage(nc, tc, b, head, typ, x, e_sb, ones97, ones1, Wsb, Bsb, Bflat,
                biasmat, kq_ps, vt_ps, sc_ps, s2_ps, av_ps, kq_sb, vt_sbp, sm_sb):
    base = 0 if typ == 0 else 3
    m_k, m_q, m_v = (base + 0) * 2 + head, (base + 1) * 2 + head, (base + 2) * 2 + head

    def xslice(ci, s0, ns):
        """x positions for seqs [s0, s0+ns): AP [128, ns, 96] (seq-major)."""
        if typ == 0:
            return x[:, ci, s0:s0 + ns, :]
        xs = x[:, ci, :, s0:s0 + ns]          # free dims: (r=96, c=ns)
        return _reap(xs, [1, 0])              # -> (c=ns, r=96)

    for c0 in range(NCHUNKS):
        s_base = c0 * CHUNK
        k_t = kq_sb.tile([128, 2, CHUNK, NPOS], BF16, tag="k_sb")
        q_t = kq_sb.tile([128, 2, CHUNK, NPOS], BF16, tag="q_sb")
        # ---- k, q projections ----
        for (m_idx, dst, eng) in ((m_k, k_t, 0), (m_q, q_t, 1)):
            for co in range(2):
                for f in range(CHUNK // SUB):
                    ps = kq_ps.tile([128, SUB, NPOS], F32, tag="kq_ps")
                    for ci in range(2):
                        nc.tensor.matmul(
                            ps, _f32r(Wsb[:, m_idx, ci, co * 128:(co + 1) * 128]),
                            _f32r(xslice(ci, s_base + f * SUB, SUB)),
                            start=(ci == 0), stop=(ci == 1))
                    dstv = dst[:, co, f * SUB:(f + 1) * SUB, :]
                    bias = Bsb[:, m_idx, co:co + 1]
                    if eng == 0:
                        nc.vector.tensor_scalar_add(out=dstv, in0=ps, scalar1=bias)
                    else:
                        nc.vector.tensor_scalar_add(out=dstv, in0=ps, scalar1=bias)
        # ---- v^T projection (per seq, x stationary), bias via K=1 matmul ----
        vt_t = vt_sbp.tile([96, CHUNK, 256], BF16, tag="vt_sb")
        for j in range(CHUNK // 2):
            ps = vt_ps.tile([96, 2, 512], F32, tag="vt_ps")
            for jj in range(2):
                s = s_base + j * 2 + jj
                for ci in range(2):
                    nc.tensor.matmul(
                        ps[:, jj, 0:256], _f32r(xslice(ci, s, 1)[:, 0, :]),
                        _f32r(Wsb[:, m_v, ci, :]),
                        start=(ci == 0), stop=False)
                nc.tensor.matmul(
                    ps[:, jj, 0:256], _f32r(ones1[0:1, 0:96]),
                    _f32r(Bflat[:, m_v * 256:(m_v + 1) * 256]),
                    start=False, stop=True)
            nc.scalar.activation(
                out=vt_t[:, j * 2:j * 2 + 2, :],
                in_=ps[:, :, 0:256],
                func=AF.Copy)
        # ---- attention ----
        if K_PHASE == "proj":
            continue
        for g in range(CHUNK // 2):
            s0 = s_base + g * 2
            for P in range(2):
                grp = ((c0 * (CHUNK // 2) + g) * 2 + P) % 4
                slots = (grp * 2, grp * 2 + 1)
                for si in range(2):
                    s = s0 + si
                    slot = slots[si]
                    sc = sc_ps.tile([96, 2, 512], F32, tag="sc_ps")
                    for hh in range(2):  # row-packed head pair
                        p0 = hh * 64
                        nc.tensor.matmul(
                            sc[:, hh, 0:96],
                            q_t[p0:p0 + 64, P, s - s_base, :],
                            k_t[p0:p0 + 64, P, s - s_base, :],
                            start=True, stop=True)
                    e_out = e_sb[0:96, slot, :].rearrange("p (a c) -> p a c", a=2)
                    nc.scalar.activation(
                        out=e_out, in_=sc[:, :, 0:96],
                        func=AF.Exp, scale=0.125,
                        bias=biasmat[:, b, typ, s:s + 1])
                # S: col-packed pair, both seqs of the group in one bank
                S2 = s2_ps.tile([128, 2, 96], F32, tag="s2_ps")
                e_pair = e_sb[:, slots[0]:slots[0] + 2, :]   # [97, 2, 192]
                nc.tensor.matmul(S2[0:64, :, :], ones97[:, 0:64],
                                 e_pair[:, :, 0:96],
                                 start=True, stop=True, tile_position=(0, 0))
                nc.tensor.matmul(S2[64:128, :, :], ones97[:, 64:128],
                                 e_pair[:, :, 96:192],
                                 start=True, stop=True, tile_position=(0, 64))
                # AV: per seq, col-packed head pair
                AV = av_ps.tile([128, 2, 512], F32, tag="av_ps")
                for si in range(2):
                    s = s0 + si
                    slot = slots[si]
                    for hh in range(2):
                        cp = hh * 64
                        nc.tensor.matmul(
                            AV[cp:cp + 64, si, 0:96],
                            vt_t[:, s - s_base, P * 128 + cp:P * 128 + cp + 64],
                            e_sb[0:96, slot, hh * 96:(hh + 1) * 96],
                            start=True, stop=True, tile_position=(0, cp))
                # normalize + residual
                R = sm_sb.tile([128, 2, 96], F32, tag="r_sb")
                nc.vector.reciprocal(out=R, in_=S2)
                D = sm_sb.tile([128, 2, 96], F32, tag="d_sb")
                nc.vector.scalar_tensor_tensor(
                    out=D, in0=AV[:, :, 0:96], scalar=1.0, in1=R,
                    op0=ALU.mult, op1=ALU.mult)
                if K_PHASE == "attn":
                    continue
                if typ == 0:
                    xs = x[:, P, s0:s0 + 2, :]
                else:
                    xs = _reap(x[:, P, :, s0:s0 + 2], [1, 0])
                if K_RESID == "gpsimd":
                    nc.gpsimd.tensor_add(out=xs, in0=xs, in1=D)
                else:
                    nc.vector.tensor_add(out=xs, in0=xs, in1=D)


def _host_pack(inputs):
    """Pack weights/bias/biasmat host arrays; returns per-core input maps."""
    feats = np.ascontiguousarray(inputs["feats"], dtype=np.float32)
    num_rows = np.asarray(inputs["num_rows"]).astype(np.int64)
    num_cols = np.asarray(inputs["num_cols"]).astype(np.int64)
    w_list, b_list = [], []
    for nm in NAMES:
        for h in range(2):
            w_list.append(np.ascontiguousarray(
                np.asarray(inputs[nm + "_w"][h], dtype=np.float32).T))
            b_list.append(np.asarray(inputs[nm + "_b"][h], dtype=np.float32))
    w_all = np.stack(w_list)                     # [12, cin, cout]
    b_all = np.concatenate([np.stack(b_list),
                            np.ones((1, 256), np.float32)])  # [13, 256], row 12 = ones

    in_maps = []
    for core in range(NCORES):
        bsl = slice(core * BLOCAL, (core + 1) * BLOCAL)
        nr_ = num_rows[bsl]; nc_ = num_cols[bsl]
        bm = np.full((BLOCAL, 2, 96, 96), -30000.0, np.float32)
        for i in range(BLOCAL):
            cm = np.arange(96) < nc_[i]
            rm = np.arange(96) < nr_[i]
            # col-type: m=c(pos), seq=r
            bm[i, 0][np.ix_(cm, rm)] = 0.0
            # row-type: m=r(pos), seq=c
            bm[i, 1][np.ix_(rm, cm)] = 0.0
        in_maps.append(dict(feats=feats[bsl], w_all=w_all, b_all=b_all, bm_in=bm))
    return in_maps


def kernel(**inputs):
    global _BUILT
    if _BUILT is None:
        _BUILT = _build()
    nc = _BUILT
    in_maps = _host_pack(inputs)
    res = run_bass_kernel_spmd(nc, in_maps, core_ids=list(range(NCORES)))
    x_full = np.concatenate([r["out"] for r in res.results], axis=0)

    num_rows = np.asarray(inputs["num_rows"])
    num_cols = np.asarray(inputs["num_cols"])
    rmask = np.arange(NR)[None, :] < num_rows[:, None]
    cmask = np.arange(NC)[None, :] < num_cols[:, None]
    masks = (rmask[:, :, None] & cmask[:, None, :]).astype(np.float32)
    return x_full.astype(np.float32), masks


# revision 11
# speedup vs baseline: 1.0290x; 1.0290x over previous
"""Trainium2 Bass kernel for nn_CellsExtractor (axial attention over ragged grid).

16 batches data-parallel over 8 cores (2 per core, processed sequentially,
x resident in SBUF). Per batch, 4 stages: col-attn(h0), row-attn(h0),
col-attn(h1), row-attn(h1). Each stage = 96 sequences x 96 positions,
C=256, 4 heads of D=64.

Per stage:
  k,q: fp32r projections (weights stationary), psum->sbuf bf16 copies with
       per-partition bias (k on ACT-diverted DVE path, q on DVE).
  v^T: per-seq matmuls with x-slice stationary -> [pos, c] layout directly;
       v-bias folded in via a K=1 accumulating matmul; plain ACT Copy out.
  scores^T (lhsT=q_h, rhs=k_h) -> [m,n] psum, heads row-packed (0/64),
       bank-aligned psum slots.
  e = exp(scores/8 + maskbias[m]) via one ACT op per (seq, head-pair),
       written bf16 into a ring tile with a 97th eps row (1e-30).
  S (softmax denominators broadcast to all partitions) via ones-matrix
       matmuls, col-packed head pair, 2 seqs per bank.
  AV (lhsT = v^T head columns, rhs = e) -> [c, n] psum, col-packed pair.
  D = AV * reciprocal(S); x += D on GPSIMD.
Masking via host-precomputed biasmat[m, seq] (0 / -30000); fully-masked
sequences give e=0 everywhere -> D = 0/eps = 0 -> x unchanged (matches ref).
"""
import os
import numpy as np
import ml_dtypes

import concourse.bass as bass
from concourse import bacc
import concourse.tile as tile
from concourse import mybir
from concourse.bass_utils import run_bass_kernel_spmd

F32 = mybir.dt.float32
BF16 = mybir.dt.bfloat16
F32R = mybir.dt.float32r
AF = mybir.ActivationFunctionType
ALU = mybir.AluOpType

BS, C, NR, NC = 16, 256, 96, 96
NCORES = 8
BLOCAL = BS // NCORES
NSEQ, NPOS = 96, 96
CHUNK = 16
NCHUNKS = NSEQ // CHUNK
SUB = 4                       # seqs per projection matmul (N = 4*96 = 384)
STAGES = [(0, 0), (0, 1), (1, 0), (1, 1)]   # (head, typ); typ 0=col, 1=row
NAMES = ["col_k", "col_q", "col_v", "row_k", "row_q", "row_v"]

_BUILT = None
K_PHASE = os.environ.get("K_PHASE", "full")   # dma | proj | attn | full
K_RESID = os.environ.get("K_RESID", "gpsimd") # gpsimd | dve
K_NSTAGE = int(os.environ.get("K_NSTAGE", "8"))


def _f32r(ap):
    return ap.bitcast(F32R)


def _reap(ap, dims):
    """Reorder/select free dims of an AP: dims are indices into ap.ap[1:]."""
    return bass.AP(tensor=ap.tensor, offset=ap.offset,
                   ap=[ap.ap[0]] + [ap.ap[1 + d] for d in dims])


def _build():
    nc = bacc.Bacc()
    feats = nc.dram_tensor("feats", [BLOCAL, C, NR, NC], F32, kind="ExternalInput")
    w_all = nc.dram_tensor("w_all", [12, C, C], F32, kind="ExternalInput")
    b_all = nc.dram_tensor("b_all", [13, C], F32, kind="ExternalInput")
    bm_in = nc.dram_tensor("bm_in", [BLOCAL, 2, 96, 96], F32, kind="ExternalInput")
    out = nc.dram_tensor("out", [BLOCAL, C, NR, NC], F32, kind="ExternalOutput")

    with tile.TileContext(nc) as tc:
        # ---- persistent sbuf tensors ----
        x = nc.alloc_sbuf_tensor("x", [128, 2, NR, NC], F32R).ap()
        e_sb = nc.alloc_sbuf_tensor("e_sb", [97, 8, 192], BF16).ap()
        ones97 = nc.alloc_sbuf_tensor("ones97", [97, 128], BF16).ap()
        Wsb = nc.alloc_sbuf_tensor("Wsb", [128, 12, 2, 256], F32R).ap()
        Bsb = nc.alloc_sbuf_tensor("Bsb", [128, 12, 2], F32).ap()
        Bflat = nc.alloc_sbuf_tensor("Bflat", [1, 13 * 256], F32R).ap()
        biasmat = nc.alloc_sbuf_tensor("biasmat", [96, BLOCAL, 2, 96], F32).ap()

        nc.vector.memset(e_sb[96:97, :, :], 1e-30)
        nc.vector.memset(ones97[:, :], 1.0)

        # weights: w_all[m] is pre-transposed on host to [cin, cout]
        nc.gpsimd.dma_start(
            out=Wsb,
            in_=bass.AP(tensor=w_all, offset=0,
                        ap=[[256, 128], [65536, 12], [128 * 256, 2], [1, 256]]))
        with nc.allow_non_contiguous_dma(reason="3KB bias load, once"):
            nc.sync.dma_start(
                out=Bsb,
                in_=bass.AP(tensor=b_all, offset=0,
                            ap=[[1, 128], [256, 12], [128, 2]]))
        nc.gpsimd.dma_start(
            out=Bflat,
            in_=bass.AP(tensor=b_all, offset=0, ap=[[3328, 1], [1, 3328]]))
        nc.sync.dma_start(
            out=biasmat,
            in_=bass.AP(tensor=bm_in, offset=0,
                        ap=[[96, 96], [2 * 96 * 96, BLOCAL], [96 * 96, 2], [1, 96]]))

        with (
            tc.tile_pool(name="kq_ps", bufs=1, space="PSUM") as kq_ps,
            tc.tile_pool(name="vt_ps", bufs=1, space="PSUM") as vt_ps,
            tc.tile_pool(name="sc_ps", bufs=1, space="PSUM") as sc_ps,
            tc.tile_pool(name="s2_ps", bufs=1, space="PSUM") as s2_ps,
            tc.tile_pool(name="av_ps", bufs=1, space="PSUM") as av_ps,
            tc.tile_pool(name="kq_sb", bufs=2) as kq_sb,
            tc.tile_pool(name="vt_sb", bufs=2) as vt_sbp,
            tc.tile_pool(name="sm_sb", bufs=4) as sm_sb,
        ):
            for b in range(BLOCAL):
                # load x for this batch
                nc.gpsimd.dma_start(
                    out=x,
                    in_=bass.AP(tensor=feats, offset=b * C * NR * NC,
                                ap=[[NR * NC, 128], [128 * NR * NC, 2],
                                    [NC, NR], [1, NC]]))
                for st, (head, typ) in enumerate(STAGES):
                    if b * 4 + st >= K_NSTAGE or K_PHASE == "dma":
                        continue
                    ones1 = Bflat[0:1, 12 * 256:12 * 256 + 128]
                    _emit_stage(nc, tc, b, head, typ,
                                x, e_sb, ones97, ones1, Wsb, Bsb, Bflat, biasmat,
                                kq_ps, vt_ps, sc_ps, s2_ps, av_ps,
                                kq_sb, vt_sbp, sm_sb)
                nc.sync.dma_start(
                    out=bass.AP(tensor=out, offset=b * C * NR * NC,
                                ap=[[NR * NC, 128], [128 * NR * NC, 2],
                                    [NC, NR], [1, NC]]),
                    in_=x.bitcast(F32))
    nc.finalize()
    return nc


def _emit_stage(nc, tc, b, head, typ, x, e_sb, ones97, ones1, Wsb, Bsb, Bflat,
                biasmat, kq_ps, vt_ps, sc_ps, s2_ps, av_ps, kq_sb, vt_sbp, sm_sb):
    base = 0 if typ == 0 else 3
    m_k, m_q, m_v = (base + 0) * 2 + head, (base + 1) * 2 + head, (base + 2) * 2 + head

    def xslice(ci, s0, ns):
        """x positions for seqs [s0, s0+ns): AP [128, ns, 96] (seq-major)."""
        if typ == 0:
            return x[:, ci, s0:s0 + ns, :]
        xs = x[:, ci, :, s0:s0 + ns]          # free dims: (r=96, c=ns)
        return _reap(xs, [1, 0])              # -> (c=ns, r=96)

    for c0 in range(NCHUNKS):
        s_base = c0 * CHUNK
        k_t = kq_sb.tile([128, 2, CHUNK, NPOS], BF16, tag="k_sb")
        q_t = kq_sb.tile([128, 2, CHUNK, NPOS], BF16, tag="q_sb")
        # ---- k, q projections ----
        for (m_idx, dst, eng) in ((m_k, k_t, 0), (m_q, q_t, 1)):
            for co in range(2):
                for f in range(CHUNK // SUB):
                    ps = kq_ps.tile([128, SUB, NPOS], F32, tag="kq_ps")
                    for ci in range(2):
                        nc.tensor.matmul(
                            ps, _f32r(Wsb[:, m_idx, ci, co * 128:(co + 1) * 128]),
                            _f32r(xslice(ci, s_base + f * SUB, SUB)),
                            start=(ci == 0), stop=(ci == 1))
                    dstv = dst[:, co, f * SUB:(f + 1) * SUB, :]
                    bias = Bsb[:, m_idx, co:co + 1]
                    if eng == 0:
                        nc.vector.tensor_scalar_add(out=dstv, in0=ps, scalar1=bias)
                    else:
                        nc.vector.tensor_scalar_add(out=dstv, in0=ps, scalar1=bias)
        # ---- v^T projection (per seq, x stationary), bias via K=1 matmul ----
        vt_t = vt_sbp.tile([96, CHUNK, 256], BF16, tag="vt_sb")
        for j in range(CHUNK // 2):
            ps = vt_ps.tile([96, 2, 512], F32, tag="vt_ps")
            for jj in range(2):
                s = s_base + j * 2 + jj
                for ci in range(2):
                    nc.tensor.matmul(
                        ps[:, jj, 0:256], _f32r(xslice(ci, s, 1)[:, 0, :]),
                        _f32r(Wsb[:, m_v, ci, :]),
                        start=(ci == 0), stop=False)
                nc.tensor.matmul(
                    ps[:, jj, 0:256], _f32r(ones1[0:1, 0:96]),
                    _f32r(Bflat[:, m_v * 256:(m_v + 1) * 256]),
                    start=False, stop=True)
            nc.scalar.activation(
                out=vt_t[:, j * 2:j * 2 + 2, :],
                in_=ps[:, :, 0:256],
                func=AF.Copy)
        # ---- attention ----
        if K_PHASE == "proj":
            continue
        for g in range(CHUNK // 2):
            s0 = s_base + g * 2
            D4 = sm_sb.tile([128, 2, 2, 96], F32, tag="d_sb")
            for P in range(2):
                grp = ((c0 * (CHUNK // 2) + g) * 2 + P) % 4
                slots = (grp * 2, grp * 2 + 1)
                for si in range(2):
                    s = s0 + si
                    slot = slots[si]
                    sc = sc_ps.tile([96, 2, 512], F32, tag="sc_ps")
                    for hh in range(2):  # row-packed head pair
                        p0 = hh * 64
                        nc.tensor.matmul(
                            sc[:, hh, 0:96],
                            q_t[p0:p0 + 64, P, s - s_base, :],
                            k_t[p0:p0 + 64, P, s - s_base, :],
                            start=True, stop=True)
                    e_out = e_sb[0:96, slot, :].rearrange("p (a c) -> p a c", a=2)
                    nc.scalar.activation(
                        out=e_out, in_=sc[:, :, 0:96],
                        func=AF.Exp, scale=0.125,
                        bias=biasmat[:, b, typ, s:s + 1])
                # S: col-packed pair, both seqs of the group in one bank
                S2 = s2_ps.tile([128, 2, 96], F32, tag="s2_ps")
                e_pair = e_sb[:, slots[0]:slots[0] + 2, :]   # [97, 2, 192]
                nc.tensor.matmul(S2[0:64, :, :], ones97[:, 0:64],
                                 e_pair[:, :, 0:96],
                                 start=True, stop=True, tile_position=(0, 0))
                nc.tensor.matmul(S2[64:128, :, :], ones97[:, 64:128],
                                 e_pair[:, :, 96:192],
                                 start=True, stop=True, tile_position=(0, 64))
                # AV: per seq, col-packed head pair
                AV = av_ps.tile([128, 2, 512], F32, tag="av_ps")
                for si in range(2):
                    s = s0 + si
                    slot = slots[si]
                    for hh in range(2):
                        cp = hh * 64
                        nc.tensor.matmul(
                            AV[cp:cp + 64, si, 0:96],
                            vt_t[:, s - s_base, P * 128 + cp:P * 128 + cp + 64],
                            e_sb[0:96, slot, hh * 96:(hh + 1) * 96],
                            start=True, stop=True, tile_position=(0, cp))
                # normalize + residual
                R = sm_sb.tile([128, 2, 96], F32, tag="r_sb")
                nc.vector.reciprocal_approx_fast(out=R, in_=S2)
                nc.vector.scalar_tensor_tensor(
                    out=D4[:, P, :, :], in0=AV[:, :, 0:96], scalar=1.0, in1=R,
                    op0=ALU.mult, op1=ALU.mult)
            if K_PHASE == "attn":
                continue
            if typ == 0:
                xs = x[:, :, s0:s0 + 2, :]
            else:
                xs = _reap(x[:, :, :, s0:s0 + 2], [0, 2, 1])
            if K_RESID == "gpsimd":
                nc.gpsimd.tensor_add(out=xs, in0=xs, in1=D4)
            else:
                nc.vector.tensor_add(out=xs, in0=xs, in1=D4)


def _host_pack(inputs):
    """Pack weights/bias/biasmat host arrays; returns per-core input maps."""
    feats = np.ascontiguousarray(inputs["feats"], dtype=np.float32)
    num_rows = np.asarray(inputs["num_rows"]).astype(np.int64)
    num_cols = np.asarray(inputs["num_cols"]).astype(np.int64)
    w_list, b_list = [], []
    for nm in NAMES:
        for h in range(2):
            w_list.append(np.ascontiguousarray(
                np.asarray(inputs[nm + "_w"][h], dtype=np.float32).T))
            b_list.append(np.asarray(inputs[nm + "_b"][h], dtype=np.float32))
    w_all = np.stack(w_list)                     # [12, cin, cout]
    b_all = np.concatenate([np.stack(b_list),
                            np.ones((1, 256), np.float32)])  # [13, 256], row 12 = ones

    in_maps = []
    for core in range(NCORES):
        bsl = slice(core * BLOCAL, (core + 1) * BLOCAL)
        nr_ = num_rows[bsl]; nc_ = num_cols[bsl]
        bm = np.full((BLOCAL, 2, 96, 96), -30000.0, np.float32)
        for i in range(BLOCAL):
            cm = np.arange(96) < nc_[i]
            rm = np.arange(96) < nr_[i]
            # col-type: m=c(pos), seq=r
            bm[i, 0][np.ix_(cm, rm)] = 0.0
            # row-type: m=r(pos), seq=c
            bm[i, 1][np.ix_(rm, cm)] = 0.0
        in_maps.append(dict(feats=feats[bsl], w_all=w_all, b_all=b_all, bm_in=bm))
    return in_maps


def kernel(**inputs):
    global _BUILT
    if _BUILT is None:
        _BUILT = _build()
    nc = _BUILT
    in_maps = _host_pack(inputs)
    res = run_bass_kernel_spmd(nc, in_maps, core_ids=list(range(NCORES)))
    x_full = np.concatenate([r["out"] for r in res.results], axis=0)

    num_rows = np.asarray(inputs["num_rows"])
    num_cols = np.asarray(inputs["num_cols"])
    rmask = np.arange(NR)[None, :] < num_rows[:, None]
    cmask = np.arange(NC)[None, :] < num_cols[:, None]
    masks = (rmask[:, :, None] & cmask[:, None, :]).astype(np.float32)
    return x_full.astype(np.float32), masks
